# revision 40
# baseline (speedup 1.0000x reference)
"""Contrastive cosine-similarity softmax-CE loss on 8 trn2 NeuronCores.

reference math:
    n1 = f1 / max(||f1||, eps);  n2 = f2 / max(||f2||, eps)
    logits = (n1 @ n2.T) / TEMP                      # [8192, 8192]
    loss = mean_i( logsumexp_j(logits[i, :]) - logits[i, i] )

sharding: f1 rows data-parallel across 8 cores (1024 rows each); f2
replicated (each core streams all of f2 from its HBM copy).  Per-core
output is the vector of per-row (lse - l_ii); host averages.
(An AllGather of the normalized f2 shards was microbenched and rejected:
+88us for a 0.79MB/core gather in this runtime.)

Device-side algorithm per core (all SPMD-uniform, no collectives):
  - logits never max-subtracted: |logit| <= 1/0.07 = 14.29 by
    Cauchy-Schwarz so exp() stays in fp32 range.  Single-pass softmax.
  - the eps clamp of the reference (||f|| >= 1e-8) is a mathematical
    no-op for these inputs and is skipped.
  - f1 is NOT normalized pre-GEMM; inv-norm/TEMP rides in as the
    per-partition `scale` of the fused Exp activation (accum_out gives
    the per-row softmax denominator in the same ScalarE op).  f1t is
    pre-cast to bf16 on the host and cast bf16->fp8 inside the SWDGE
    load; f2s is likewise host-pre-cast to bf16 (halves every HBM read,
    and the stage-DMA cadence gates the startup pipe-fill).
  - f2 IS normalized pre-GEMM (inv-norm varies along the free dim),
    prescaled by SC=32 into fp8e4m3's sweet spot; 1/SC folds into the
    Exp scale.  Main GEMM runs fp8 DoubleRow (~1.45x bf16 rate).
  - per-f2-row sum-of-squares via ones-matmuls over squares; split
    asymmetrically: k-chunks 0-3 squared to fp8 (1x DVE) and summed
    with DoubleRow fp8 matmuls, chunks 4-5 squared to bf16 (2x DVE)
    and summed with bf16 matmuls -- balances DVE vs TensorE load.
  - per-core pair ROTATION (host-side): core c's local pair q holds
    global f2 block (c+q)%8, so the diagonal block is always local
    pair 0 and the program stays SPMD-uniform.  The diagonal logit is
    extracted from the pair-0 PSUM tiles with an identity-mask multiply
    (accum_out), killing the baseline's separate diag chain (6.3 MB of
    DMA + 16 DVE ops).
  - f1 row norms likewise come from ones-matmuls over squared n1t8
    (squares on the otherwise-idle Pool engine) + identity-mask extract
    (no f1n load at all).
  - EMISSION ORDER IS EXECUTION ORDER per engine (strict FIFO queues):
    prep for pair p+2 is WOVEN into main_pair(p)'s m-loop (squares
    after m0, sum-of-squares+Ln/Exp after m3, normalize after m5) so
    no engine's queue ever blocks a ready op behind a not-ready one.
    A plain "emit prep(p+2) then main(p)" serializes pair-p EXPs
    behind prep-(p+2)'s Ln on the Scalar queue (measured: first EXP
    at 62us instead of ~29us).
  - _split_excess_waits(): Tile attaches more sync waits per
    instruction than the 64B TPB encodings carry; excess waits are
    hoisted into standalone InstEventSemaphore on the same engine.
"""

import sys

for _p in ("/opt/trn_rl_repo",):
    if _p not in sys.path:
        sys.path.insert(0, _p)

from contextlib import ExitStack

import numpy as np

import concourse.bass as bass
import concourse.tile as tile
from concourse import mybir

FP32 = mybir.dt.float32
BF16 = mybir.dt.bfloat16
FP8 = mybir.dt.float8e4
AF = mybir.ActivationFunctionType
ALU = mybir.AluOpType
AX = mybir.AxisListType

N = 8192        # rows of f1/f2
D = 768         # feature dim
NCORES = 8
MC = N // NCORES        # f1 rows per core (1024)
KT = D // 128           # contraction k-chunks (6)
KT8 = 4                 # k-chunks squared to fp8 (DoubleRow ss path)
MT = MC // 128          # f1 row tiles per core (8)
PAIR = 1024             # f2 rows processed per outer step
NPAIR = N // PAIR       # 8
NLEG = 4                # pairs 0-3 per-pair; 4-7 as two wide groups
QCOL = NLEG + 2         # spart cols per m
TEMP = 0.07
SC = 32.0               # power-of-2 prescale for the fp8 f2 operand
LOG_INV_TEMP = float(-np.log(TEMP))

_WAIT_SPLIT_SKIP = (
    "InstEventSemaphore",
    "InstHalt",
)


def _split_excess_waits(nc: bass.Bass, cap: int = 1) -> None:
    """Hoist per-instruction sync waits beyond `cap` into standalone
    InstEventSemaphore instructions on the same engine."""
    n = 0
    for bb in nc.main_func.blocks:
        new_list = []
        for inst in bb.instructions:
            si = inst.sync_info
            ow = list(si.on_wait) if si is not None and si.on_wait else []
            if len(ow) > cap and type(inst).__name__ not in _WAIT_SPLIT_SKIP:
                excess, keep = ow[:-cap], ow[-cap:]
                for w in excess:
                    n += 1
                    ev = mybir.InstEventSemaphore(
                        name=f"I-waitsplit-{n}",
                        engine=inst.engine,
                        ins=[],
                        outs=[],
                        sync_info=mybir.SyncInfo(on_wait=[w], on_update=[]),
                    )
                    nc.register_instruction(ev)
                    new_list.append(ev)
                si.on_wait = keep
            new_list.append(inst)
        bb.instructions[:] = new_list


def build_program() -> bass.Bass:
    nc = bass.Bass()
    f1t = nc.declare_dram_parameter("f1t", [D, MC], BF16, isOutput=False)
    f2s = nc.declare_dram_parameter(
        "f2s", [NPAIR, 128, KT, PAIR], BF16, isOutput=False
    )
    eye = nc.declare_dram_parameter("eye", [128, 128], FP32, isOutput=False)
    out = nc.declare_dram_parameter("out", [128, MT], FP32, isOutput=True)

    with tile.TileContext(nc, pool_alloc_mode="queue") as tc, ExitStack() as ctx:
        singles = ctx.enter_context(tc.tile_pool(name="singles", bufs=1))

        # fp8 ones for DoubleRow ss matmuls; bf16 ones for the k4/k5 path
        ones8 = singles.tile([128, 2, 128], FP8, tag="ones8", name="ones8")
        nc.vector.memset(ones8[:], 1.0)
        onesb = singles.tile([128, 128], BF16, tag="onesb", name="onesb")
        nc.vector.memset(onesb[:], 1.0)
        # exp scale bias: ln(1/TEMP) - ln(SC)  (SC un-scales the fp8 prescale)
        lbias = singles.tile([128, 1], FP32, tag="lbias", name="lbias")
        nc.vector.memset(lbias[:], LOG_INV_TEMP - float(np.log(SC)))
        # invn2 bias: +ln(SC) prescales normalized f2 into fp8's sweet spot
        sbias = singles.tile([128, 1], FP32, tag="sbias", name="sbias")
        nc.vector.memset(sbias[:], float(np.log(SC)))

        eyeb = singles.tile([128, 128], BF16, tag="eyeb", name="eyeb")

        # warm the natural_log_exp ACT table set (~2.7us load) while the
        # Scalar engine is otherwise idle waiting for the first stage
        warml = singles.tile([128, 1], FP32, tag="warml", name="warml")
        nc.scalar.activation(warml[:], lbias[:], AF.Ln)

        n1t8 = singles.tile([128, KT, MC], FP8, tag="n1t8", name="n1t8")
        invn1T = singles.tile([128, MT], FP32, tag="invn1T", name="invn1T")
        draw = singles.tile([128, MT], FP32, tag="draw", name="draw")
        dvals = singles.tile([128, MT], FP32, tag="dvals", name="dvals")
        ss1x = singles.tile([128, MT], FP32, tag="ss1x", name="ss1x")
        spart = singles.tile([128, MT * QCOL], FP32, tag="spart", name="spart")

        n2p = ctx.enter_context(tc.tile_pool(name="n2p", bufs=1))
        n2f8 = n2p.tile([128, KT, N], FP8, tag="n2f8", name="n2f8")
        stg = ctx.enter_context(tc.tile_pool(name="stg", bufs=5))
        p1p = ctx.enter_context(tc.tile_pool(name="p1p", bufs=1))
        sq1 = p1p.tile([128, KT, MC], FP8, tag="sq1", name="sq1")

        # ---- hoisted loads in consumption-priority order on the single
        # SWDGE queue (FIFO at ring granularity): pair-0 k-chunks
        # interleaved with the f1t->fp8 cast chunks (both gate the first
        # GEMM + the invn1T chain), then pairs 1..7 whole.
        stages = []
        for pair in range(NPAIR):
            st = stg.tile([128, KT, PAIR], BF16, tag="stage", name="stage")
            stages.append(st)
        # f1t chunks FIRST: the invn1T chain (sq1 -> ss1 -> extract -> ln/exp)
        # gates every EXP; its data must not queue behind the f2 stream.
        # eye's dispatch goes AFTER the critical chunk stream (eyeb is first
        # needed by the extracts at ~14us; ahead of f1t it cost 0.64us of
        # dispatch serialization on everything).
        for k in range(KT):
            nc.gpsimd.dma_start(n1t8[:, k, :], f1t[k * 128 : (k + 1) * 128, :])
        for k in range(KT):
            nc.gpsimd.dma_start(stages[0][:, k, :], f2s[0, :, k, :])
        nc.gpsimd.dma_start(eyeb[:], eye[:, :])
        # only pairs 1..3 upfront: a gated dma dispatch at the head of the
        # GpSimd FIFO blocks everything emitted after it; pairs 4..7 are
        # emitted right after the prep_norm that frees their buffer.
        for pair in range(1, 4):
            nc.gpsimd.dma_start(stages[pair][:], f2s[pair])

        with tc.tile_pool(name="wk", bufs=3) as wp, tc.tile_pool(
            name="psl", bufs=2, space="PSUM"
        ) as pl, tc.tile_pool(name="expp", bufs=3) as ep:
            sqtiles = {}
            invn2s = {}

            def prep_sq(pair, pool_chunks=0):
                # squares: chunks 0..3 -> fp8 (1x), chunks 4,5 -> bf16 (2x);
                # pair 0 offloads its first fp8 chunks to the idle Pool to
                # shorten the head-critical DVE chain
                st = stages[pair]
                sq8 = wp.tile([128, KT8, PAIR], FP8, tag="sq8", name="sq8")
                sqb = wp.tile([128, KT - KT8, PAIR], BF16, tag="sqb", name="sqb")
                for k in range(KT):
                    if k < KT8:
                        eng = nc.gpsimd if k < pool_chunks else nc.vector
                        eng.tensor_mul(sq8[:, k, :], st[:, k, :], st[:, k, :])
                    else:
                        nc.vector.tensor_mul(
                            sqb[:, k - KT8, :], st[:, k, :], st[:, k, :]
                        )
                sqtiles[pair] = (sq8, sqb)

            def f1_sq():
                # f1 squares, all DVE: the Pool ENGINE queue is clogged at
                # t=0 by ~16 serial DMA-dispatch ops (~0.64us each), so
                # pool-side squares started only at ~10us and pushed the
                # whole f1 -> invn1T chain (and PE's first matmul) to ~15us
                for k in range(KT):
                    nc.vector.tensor_mul(
                        sq1[:, k, :], n1t8[:, k, :], n1t8[:, k, :]
                    )

            def prep_ss(pair):
                # per-f2-row sum of squares broadcast to all 128 partitions
                sq8, sqb = sqtiles.pop(pair)
                sst = pl.tile([128, 2048], FP32, tag="pslog", name="ss")
                ss = sst[:, 0:PAIR]
                for jk in range(KT8 // 2):
                    for h in range(2):
                        nc.tensor.matmul(
                            ss[:, h * 512 : (h + 1) * 512],
                            ones8[:],
                            sq8[:, 2 * jk : 2 * jk + 2, h * 512 : (h + 1) * 512],
                            start=(jk == 0),
                            stop=False,
                            perf_mode=mybir.MatmulPerfMode.DoubleRow,
                        )
                for k in range(KT8, KT):
                    for h in range(2):
                        nc.tensor.matmul(
                            ss[:, h * 512 : (h + 1) * 512],
                            onesb[:],
                            sqb[:, k - KT8, h * 512 : (h + 1) * 512],
                            start=False,
                            stop=(k == KT - 1),
                        )
                lntmp = wp.tile([128, PAIR], FP32, tag="lntmp", name="lntmp", bufs=1)
                nc.scalar.activation(lntmp[:], ss[:], AF.Ln)
                # invn2 = SC / ||f2_j||
                invn2 = wp.tile([128, PAIR], BF16, tag="invn2", name="invn2")
                nc.scalar.activation(
                    invn2[:], lntmp[:], AF.Exp, scale=-0.5, bias=sbias[:]
                )
                invn2s[pair] = invn2

            def prep_norm(pair):
                # normalized+prescaled chunk into the resident fp8 tile
                c0 = pair * PAIR
                st = stages[pair]
                invn2 = invn2s.pop(pair)
                for k in range(KT):
                    nc.vector.tensor_mul(
                        n2f8[:, k, c0 : c0 + PAIR], st[:, k, :], invn2[:]
                    )
                # stage pair+4 reuses this pair's buffer: emit its load here
                # so the dispatch op enters the GpSimd FIFO only when it can
                # actually run (see note at the hoisted loads)
                if pair + 4 < NPAIR:
                    nc.gpsimd.dma_start(stages[pair + 4][:], f2s[pair + 4])

            def f1_norms_mm():
                # ss1 = ||f1_i||^2 via DoubleRow ones-matmuls over squared
                # n1t8 (broadcast over partitions); extract split out below
                ss1t = pl.tile([128, 2048], FP32, tag="pslog", name="ss1bc")
                ss1bc = ss1t[:, 0:MC]
                for jk in range(KT // 2):
                    for h in range(2):
                        nc.tensor.matmul(
                            ss1bc[:, h * 512 : (h + 1) * 512],
                            ones8[:],
                            sq1[:, 2 * jk : 2 * jk + 2, h * 512 : (h + 1) * 512],
                            start=(jk == 0),
                            stop=(jk == KT // 2 - 1),
                            perf_mode=mybir.MatmulPerfMode.DoubleRow,
                        )
                return ss1bc

            def f1_extract(ss1bc):
                # identity-mask extracts + Ln/Exp -> invn1T, emitted AFTER
                # pair-0's prep so they sit behind sq0/norm0 on the DVE FIFO
                for m in range(MT):
                    junk = ep.tile([128, 128], BF16, tag="junk", name="junk", bufs=2)
                    nc.vector.scalar_tensor_tensor(
                        junk[:],
                        ss1bc[:, m * 128 : (m + 1) * 128],
                        1.0,
                        eyeb[:],
                        op0=ALU.mult,
                        op1=ALU.mult,
                        accum_out=ss1x[:, m : m + 1],
                    )
                t1 = ep.tile([128, MT], FP32, tag="t1", name="t1", bufs=1)
                nc.scalar.activation(t1[:], ss1x[:], AF.Ln)
                # invn1T = 1 / (||f1_i|| * TEMP * SC)
                nc.scalar.activation(
                    invn1T[:], t1[:], AF.Exp, scale=-0.5, bias=lbias[:]
                )

            def main_pair(pair, weave=None, diag=False):
                # weave: {m_index: callback} emitted after that m's EXP so
                # every engine's FIFO queue stays in executable order
                c0 = pair * PAIR
                for m in range(MT):
                    ps2 = pl.tile([128, 2048], FP32, tag="pslog", name="pslog")
                    pslog = ps2[:, 0:PAIR]
                    for jk in range(KT // 2):
                        for h in range(2):
                            nc.tensor.matmul(
                                pslog[:, h * 512 : (h + 1) * 512],
                                n1t8[:, 2 * jk : 2 * jk + 2, m * 128 : (m + 1) * 128],
                                n2f8[
                                    :,
                                    2 * jk : 2 * jk + 2,
                                    c0 + h * 512 : c0 + (h + 1) * 512,
                                ],
                                start=(jk == 0),
                                stop=(jk == KT // 2 - 1),
                                perf_mode=mybir.MatmulPerfMode.DoubleRow,
                            )
                    # in-place EXP into the PSUM tile (exponentials are
                    # discarded; only accum_out matters) - saves the eb
                    # SBUF write traffic
                    col = m * QCOL + pair
                    nc.scalar.activation(
                        pslog[:],
                        pslog[:],
                        AF.Exp,
                        scale=invn1T[:, m : m + 1],
                        accum_out=spart[:, col : col + 1],
                    )
                    if diag:
                        # diagonal block of this core's rows lives in local
                        # pair 0 (host-side rotation): mask-extract the
                        # diagonal EXPONENTIAL (post-in-place-EXP; finalize
                        # takes Ln).  Extracting AFTER the EXP keeps the EXP
                        # free of any DVE dependency: Tile lowers cross-
                        # engine deps to per-engine counting semaphores, so
                        # an EXP->diag WAR made every EXP wait for ALL
                        # earlier DVE work incl. later pairs' prep
                        # (measured: first EXP at 58us instead of ~30us)
                        junk = ep.tile(
                            [128, 128], BF16, tag="junk", name="junk", bufs=2
                        )
                        nc.vector.scalar_tensor_tensor(
                            junk[:],
                            pslog[:, m * 128 : (m + 1) * 128],
                            1.0,
                            eyeb[:],
                            op0=ALU.mult,
                            op1=ALU.mult,
                            accum_out=draw[:, m : m + 1],
                        )
                    if weave and m in weave:
                        weave[m]()

            def main_group(pa, pb, gi, weave=None):
                # wide m-loop over two pairs: one [128,2048] ring tile and
                # ONE EXP per m covering both (halves the ACT per-op cost)
                for m in range(MT):
                    pslog = pl.tile([128, 2048], FP32, tag="pslog", name="pslog")
                    for half, pp_ in enumerate((pa, pb)):
                        hb = half * PAIR
                        c0 = pp_ * PAIR
                        for jk in range(KT // 2):
                            for h in range(2):
                                nc.tensor.matmul(
                                    pslog[:, hb + h * 512 : hb + (h + 1) * 512],
                                    n1t8[
                                        :,
                                        2 * jk : 2 * jk + 2,
                                        m * 128 : (m + 1) * 128,
                                    ],
                                    n2f8[
                                        :,
                                        2 * jk : 2 * jk + 2,
                                        c0 + h * 512 : c0 + (h + 1) * 512,
                                    ],
                                    start=(jk == 0),
                                    stop=(jk == KT // 2 - 1),
                                    perf_mode=mybir.MatmulPerfMode.DoubleRow,
                                )
                    col = m * QCOL + NLEG + gi
                    nc.scalar.activation(
                        pslog[:],
                        pslog[:],
                        AF.Exp,
                        scale=invn1T[:, m : m + 1],
                        accum_out=spart[:, col : col + 1],
                    )
                    if weave and m in weave:
                        weave[m]()

            # ---- startup: the f1-norm chain first in EVERY engine queue
            # (it gates all EXPs), then pair-0 prep.  high_priority pins it
            # ahead in the Tile scheduler's heaps: plain emission order was
            # NOT honored (t1-Ln measured at 60us, delaying every EXP).
            with tc.high_priority():
                f1_sq()
                prep_sq(0, pool_chunks=2)
                prep_ss(0)
                ss1bc = f1_norms_mm()
                prep_norm(0)
                f1_extract(ss1bc)
            # pair 0 weaves pair-1 prep and pair-2 squares; each pair's
            # ss/norm is woven one main LATER than its squares: the ss
            # ring tile's WAR then anchors its Ln's placement BEHIND the
            # preceding EXPs in the Scalar stream.  (The scheduler orders
            # streams by simulated dispatch time and its DMA model is
            # optimistic for queued stages, so earlier-woven Lns were
            # placed ahead of main0's EXPs and head-of-line blocked them:
            # first EXP measured 42.5us with deps ready at ~25us.)
            main_pair(
                0,
                diag=True,
                weave={
                    0: lambda: prep_sq(1, pool_chunks=1),
                    2: lambda: prep_ss(1),
                    3: lambda: prep_norm(1),
                    5: lambda: prep_sq(2),
                },
            )
            # draw holds exp(l_ii * s_i): dvals = ln(draw) = scaled diag
            nc.scalar.activation(dvals[:], draw[:], AF.Ln)
            main_pair(
                1,
                weave={
                    0: lambda: prep_ss(2),
                    1: lambda: prep_norm(2),
                    3: lambda: prep_sq(3),
                },
            )
            main_pair(
                2,
                weave={
                    0: lambda: prep_ss(3),
                    1: lambda: prep_norm(3),
                    2: lambda: prep_sq(4),
                    4: lambda: prep_ss(4),
                    6: lambda: prep_norm(4),
                },
            )
            main_pair(
                3,
                weave={
                    0: lambda: prep_sq(5),
                    3: lambda: prep_ss(5),
                    5: lambda: prep_norm(5),
                },
            )
            # pairs 4-7 as two wide groups.  Group (4,5) weaves preps for
            # 6 and 7; its m-slots are 3.4us apart (12 matmuls each) so the
            # ss ones-matmuls sit in the PE FIFO only after their squares
            # are provably done.
            main_group(
                4, 5, 0,
                weave={
                    0: lambda: prep_sq(6),
                    1: lambda: prep_sq(7),
                    3: lambda: prep_ss(6),
                    5: lambda: prep_norm(6),
                    6: lambda: prep_ss(7),
                    7: lambda: prep_norm(7),
                },
            )
            main_group(6, 7, 1)

            # ---- finalize ----
            S = ep.tile([128, MT], FP32, tag="S", name="S", bufs=1)
            nc.vector.reduce_sum(
                S[:], spart[:].rearrange("p (m q) -> p m q", q=QCOL), axis=AX.X
            )
            lse = ep.tile([128, MT], FP32, tag="lse", name="lse", bufs=1)
            nc.scalar.activation(lse[:], S[:], AF.Ln)
            res = ep.tile([128, MT], FP32, tag="res", name="res", bufs=1)
            nc.vector.tensor_sub(res[:], lse[:], dvals[:])
            nc.sync.dma_start(out[:, :], res[:])

    _split_excess_waits(nc)
    return nc


def make_in_maps(f1: np.ndarray, f2: np.ndarray) -> list[dict[str, np.ndarray]]:
    f1 = np.ascontiguousarray(np.asarray(f1, dtype=np.float32))
    f2 = np.ascontiguousarray(np.asarray(f2, dtype=np.float32))
    assert f1.shape == (N, D) and f2.shape == (N, D)
    f2t = f2.T  # [D, N]
    # pack pair-major, partition-major: f2s[q, p, k, n] = f2t[k*128+p, q*1024+n]
    # pre-cast to bf16 on the host: the device consumes f1/f2 at
    # bf16/fp8 precision anyway (the SWDGE load already cast fp32->bf16),
    # and halving the HBM bytes halves the stage-DMA cadence that gates
    # the startup pipe-fill (stage p landed at ~17.6+8.8p us)
    import ml_dtypes

    bf16 = ml_dtypes.bfloat16
    f2s = np.ascontiguousarray(
        f2t.reshape(KT, 128, NPAIR, PAIR).transpose(2, 1, 0, 3).astype(bf16)
    )
    eye = np.ascontiguousarray(np.eye(128, dtype=np.float32))
    in_maps = []
    for c in range(NCORES):
        f1c = np.ascontiguousarray(f1[c * MC : (c + 1) * MC])
        # rotate pairs so the diagonal block is local pair 0 on every core
        f2sc = np.ascontiguousarray(np.roll(f2s, -c, axis=0))
        in_maps.append(
            {
                "f1t": np.ascontiguousarray(f1c.T.astype(bf16)),
                "f2s": f2sc,
                "eye": eye,
            }
        )
    return in_maps


def combine_outputs(outs: list[np.ndarray]) -> np.float32:
    total = 0.0
    for o in outs:
        total += float(np.sum(np.asarray(o, dtype=np.float64)))
    return np.float32(total / float(N))


def run(f1: np.ndarray, f2: np.ndarray, trace: bool = False):
    from concourse.bass_utils import run_bass_kernel_spmd

    nc = build_program()
    in_maps = make_in_maps(f1, f2)
    r = run_bass_kernel_spmd(nc, in_maps, core_ids=list(range(NCORES)), trace=trace)
    outs = [m["out"] for m in r.results]
    return combine_outputs(outs), r


def kernel(f1: np.ndarray, f2: np.ndarray) -> np.ndarray:
    loss, _ = run(f1, f2, trace=False)
    return loss


if __name__ == "__main__":
    f1 = np.random.randn(N, D).astype(np.float32)
    f2 = np.random.randn(N, D).astype(np.float32)
    print(kernel(f1, f2))



# revision 42
# speedup vs baseline: 1.0006x; 1.0006x over previous
"""Contrastive cosine-similarity softmax-CE loss on 8 trn2 NeuronCores.

reference math:
    n1 = f1 / max(||f1||, eps);  n2 = f2 / max(||f2||, eps)
    logits = (n1 @ n2.T) / TEMP                      # [8192, 8192]
    loss = mean_i( logsumexp_j(logits[i, :]) - logits[i, i] )

sharding: f1 rows data-parallel across 8 cores (1024 rows each); f2
replicated (each core streams all of f2 from its HBM copy).  Per-core
output is the vector of per-row (lse - l_ii); host averages.
(An AllGather of the normalized f2 shards was microbenched and rejected:
+88us for a 0.79MB/core gather in this runtime.)

Device-side algorithm per core (all SPMD-uniform, no collectives):
  - logits never max-subtracted: |logit| <= 1/0.07 = 14.29 by
    Cauchy-Schwarz so exp() stays in fp32 range.  Single-pass softmax.
  - the eps clamp of the reference (||f|| >= 1e-8) is a mathematical
    no-op for these inputs and is skipped.
  - f1 is NOT normalized pre-GEMM; inv-norm/TEMP rides in as the
    per-partition `scale` of the fused Exp activation (accum_out gives
    the per-row softmax denominator in the same ScalarE op).  f1t is
    pre-cast to bf16 on the host and cast bf16->fp8 inside the SWDGE
    load; f2s is likewise host-pre-cast to bf16 (halves every HBM read,
    and the stage-DMA cadence gates the startup pipe-fill).
  - f2 IS normalized pre-GEMM (inv-norm varies along the free dim),
    prescaled by SC=32 into fp8e4m3's sweet spot; 1/SC folds into the
    Exp scale.  Main GEMM runs fp8 DoubleRow (~1.45x bf16 rate).
  - per-f2-row sum-of-squares via ones-matmuls over squares; split
    asymmetrically: k-chunks 0-3 squared to fp8 (1x DVE) and summed
    with DoubleRow fp8 matmuls, chunks 4-5 squared to bf16 (2x DVE)
    and summed with bf16 matmuls -- balances DVE vs TensorE load.
  - per-core pair ROTATION (host-side): core c's local pair q holds
    global f2 block (c+q)%8, so the diagonal block is always local
    pair 0 and the program stays SPMD-uniform.  The diagonal logit is
    extracted from the pair-0 PSUM tiles with an identity-mask multiply
    (accum_out), killing the baseline's separate diag chain (6.3 MB of
    DMA + 16 DVE ops).
  - f1 row norms likewise come from ones-matmuls over squared n1t8
    (squares on the otherwise-idle Pool engine) + identity-mask extract
    (no f1n load at all).
  - EMISSION ORDER IS EXECUTION ORDER per engine (strict FIFO queues):
    prep for pair p+2 is WOVEN into main_pair(p)'s m-loop (squares
    after m0, sum-of-squares+Ln/Exp after m3, normalize after m5) so
    no engine's queue ever blocks a ready op behind a not-ready one.
    A plain "emit prep(p+2) then main(p)" serializes pair-p EXPs
    behind prep-(p+2)'s Ln on the Scalar queue (measured: first EXP
    at 62us instead of ~29us).
  - _split_excess_waits(): Tile attaches more sync waits per
    instruction than the 64B TPB encodings carry; excess waits are
    hoisted into standalone InstEventSemaphore on the same engine.
"""

import sys

for _p in ("/opt/trn_rl_repo",):
    if _p not in sys.path:
        sys.path.insert(0, _p)

from contextlib import ExitStack

import numpy as np

import concourse.bass as bass
import concourse.tile as tile
from concourse import mybir

FP32 = mybir.dt.float32
BF16 = mybir.dt.bfloat16
FP8 = mybir.dt.float8e4
AF = mybir.ActivationFunctionType
ALU = mybir.AluOpType
AX = mybir.AxisListType

N = 8192        # rows of f1/f2
D = 768         # feature dim
NCORES = 8
MC = N // NCORES        # f1 rows per core (1024)
KT = D // 128           # contraction k-chunks (6)
KT8 = 4                 # k-chunks squared to fp8 (DoubleRow ss path)
MT = MC // 128          # f1 row tiles per core (8)
PAIR = 1024             # f2 rows processed per outer step
NPAIR = N // PAIR       # 8
NLEG = 4                # pairs 0-3 per-pair; 4-7 as two wide groups
QCOL = NLEG + 2         # spart cols per m
TEMP = 0.07
SC = 32.0               # power-of-2 prescale for the fp8 f2 operand
LOG_INV_TEMP = float(-np.log(TEMP))

_WAIT_SPLIT_SKIP = (
    "InstEventSemaphore",
    "InstHalt",
)


def _split_excess_waits(nc: bass.Bass, cap: int = 1) -> None:
    """Hoist per-instruction sync waits beyond `cap` into standalone
    InstEventSemaphore instructions on the same engine."""
    n = 0
    for bb in nc.main_func.blocks:
        new_list = []
        for inst in bb.instructions:
            si = inst.sync_info
            ow = list(si.on_wait) if si is not None and si.on_wait else []
            if len(ow) > cap and type(inst).__name__ not in _WAIT_SPLIT_SKIP:
                excess, keep = ow[:-cap], ow[-cap:]
                for w in excess:
                    n += 1
                    ev = mybir.InstEventSemaphore(
                        name=f"I-waitsplit-{n}",
                        engine=inst.engine,
                        ins=[],
                        outs=[],
                        sync_info=mybir.SyncInfo(on_wait=[w], on_update=[]),
                    )
                    nc.register_instruction(ev)
                    new_list.append(ev)
                si.on_wait = keep
            new_list.append(inst)
        bb.instructions[:] = new_list


def build_program() -> bass.Bass:
    nc = bass.Bass()
    f1t = nc.declare_dram_parameter("f1t", [D, MC], BF16, isOutput=False)
    f2s = nc.declare_dram_parameter(
        "f2s", [NPAIR, 128, KT, PAIR], BF16, isOutput=False
    )
    eye = nc.declare_dram_parameter("eye", [128, 128], FP32, isOutput=False)
    out = nc.declare_dram_parameter("out", [128, MT], FP32, isOutput=True)

    with tile.TileContext(nc, pool_alloc_mode="queue") as tc, ExitStack() as ctx:
        singles = ctx.enter_context(tc.tile_pool(name="singles", bufs=1))

        # fp8 ones for DoubleRow ss matmuls; bf16 ones for the k4/k5 path
        ones8 = singles.tile([128, 2, 128], FP8, tag="ones8", name="ones8")
        nc.vector.memset(ones8[:], 1.0)
        onesb = singles.tile([128, 128], BF16, tag="onesb", name="onesb")
        nc.vector.memset(onesb[:], 1.0)
        # exp scale bias: ln(1/TEMP) - ln(SC)  (SC un-scales the fp8 prescale)
        lbias = singles.tile([128, 1], FP32, tag="lbias", name="lbias")
        nc.vector.memset(lbias[:], LOG_INV_TEMP - float(np.log(SC)))
        # invn2 bias: +ln(SC) prescales normalized f2 into fp8's sweet spot
        sbias = singles.tile([128, 1], FP32, tag="sbias", name="sbias")
        nc.vector.memset(sbias[:], float(np.log(SC)))

        eyeb = singles.tile([128, 128], BF16, tag="eyeb", name="eyeb")

        # warm the natural_log_exp ACT table set (~2.7us load) while the
        # Scalar engine is otherwise idle waiting for the first stage
        warml = singles.tile([128, 1], FP32, tag="warml", name="warml")
        nc.scalar.activation(warml[:], lbias[:], AF.Ln)

        n1t8 = singles.tile([128, KT, MC], FP8, tag="n1t8", name="n1t8")
        invn1T = singles.tile([128, MT], FP32, tag="invn1T", name="invn1T")
        draw = singles.tile([128, MT], FP32, tag="draw", name="draw")
        dvals = singles.tile([128, MT], FP32, tag="dvals", name="dvals")
        ss1x = singles.tile([128, MT], FP32, tag="ss1x", name="ss1x")
        spart = singles.tile([128, MT * QCOL], FP32, tag="spart", name="spart")

        n2p = ctx.enter_context(tc.tile_pool(name="n2p", bufs=1))
        n2f8 = n2p.tile([128, KT, N], FP8, tag="n2f8", name="n2f8")
        stg = ctx.enter_context(tc.tile_pool(name="stg", bufs=6))
        p1p = ctx.enter_context(tc.tile_pool(name="p1p", bufs=1))
        sq1 = p1p.tile([128, KT, MC], FP8, tag="sq1", name="sq1")

        # ---- hoisted loads in consumption-priority order on the single
        # SWDGE queue (FIFO at ring granularity): pair-0 k-chunks
        # interleaved with the f1t->fp8 cast chunks (both gate the first
        # GEMM + the invn1T chain), then pairs 1..7 whole.
        stages = []
        for pair in range(NPAIR):
            st = stg.tile([128, KT, PAIR], BF16, tag="stage", name="stage")
            stages.append(st)
        # f1t chunks FIRST: the invn1T chain (sq1 -> ss1 -> extract -> ln/exp)
        # gates every EXP; its data must not queue behind the f2 stream.
        # eye's dispatch goes AFTER the critical chunk stream (eyeb is first
        # needed by the extracts at ~14us; ahead of f1t it cost 0.64us of
        # dispatch serialization on everything).
        for k in range(KT):
            nc.gpsimd.dma_start(n1t8[:, k, :], f1t[k * 128 : (k + 1) * 128, :])
        for k in range(KT):
            nc.gpsimd.dma_start(stages[0][:, k, :], f2s[0, :, k, :])
        nc.gpsimd.dma_start(eyeb[:], eye[:, :])
        # only pairs 1..3 upfront: a gated dma dispatch at the head of the
        # GpSimd FIFO blocks everything emitted after it; pairs 4..7 are
        # emitted right after the prep_norm that frees their buffer.
        for pair in range(1, 4):
            nc.gpsimd.dma_start(stages[pair][:], f2s[pair])

        with tc.tile_pool(name="wk", bufs=4) as wp, tc.tile_pool(
            name="psl", bufs=2, space="PSUM"
        ) as pl, tc.tile_pool(name="expp", bufs=3) as ep:
            sqtiles = {}
            invn2s = {}

            def prep_sq(pair, pool_chunks=0):
                # squares: chunks 0..3 -> fp8 (1x), chunks 4,5 -> bf16 (2x);
                # pair 0 offloads its first fp8 chunks to the idle Pool to
                # shorten the head-critical DVE chain
                st = stages[pair]
                sq8 = wp.tile([128, KT8, PAIR], FP8, tag="sq8", name="sq8")
                sqb = wp.tile([128, KT - KT8, PAIR], BF16, tag="sqb", name="sqb")
                for k in range(KT):
                    if k < KT8:
                        eng = nc.gpsimd if k < pool_chunks else nc.vector
                        eng.tensor_mul(sq8[:, k, :], st[:, k, :], st[:, k, :])
                    else:
                        nc.vector.tensor_mul(
                            sqb[:, k - KT8, :], st[:, k, :], st[:, k, :]
                        )
                sqtiles[pair] = (sq8, sqb)

            def f1_sq():
                # f1 squares, all DVE: the Pool ENGINE queue is clogged at
                # t=0 by ~16 serial DMA-dispatch ops (~0.64us each), so
                # pool-side squares started only at ~10us and pushed the
                # whole f1 -> invn1T chain (and PE's first matmul) to ~15us
                for k in range(KT):
                    nc.vector.tensor_mul(
                        sq1[:, k, :], n1t8[:, k, :], n1t8[:, k, :]
                    )

            def prep_ss(pair):
                # per-f2-row sum of squares broadcast to all 128 partitions
                sq8, sqb = sqtiles.pop(pair)
                sst = pl.tile([128, 2048], FP32, tag="pslog", name="ss")
                ss = sst[:, 0:PAIR]
                for jk in range(KT8 // 2):
                    for h in range(2):
                        nc.tensor.matmul(
                            ss[:, h * 512 : (h + 1) * 512],
                            ones8[:],
                            sq8[:, 2 * jk : 2 * jk + 2, h * 512 : (h + 1) * 512],
                            start=(jk == 0),
                            stop=False,
                            perf_mode=mybir.MatmulPerfMode.DoubleRow,
                        )
                for k in range(KT8, KT):
                    for h in range(2):
                        nc.tensor.matmul(
                            ss[:, h * 512 : (h + 1) * 512],
                            onesb[:],
                            sqb[:, k - KT8, h * 512 : (h + 1) * 512],
                            start=False,
                            stop=(k == KT - 1),
                        )
                lntmp = wp.tile([128, PAIR], FP32, tag="lntmp", name="lntmp", bufs=1)
                nc.scalar.activation(lntmp[:], ss[:], AF.Ln)
                # invn2 = SC / ||f2_j||
                invn2 = wp.tile([128, PAIR], BF16, tag="invn2", name="invn2")
                nc.scalar.activation(
                    invn2[:], lntmp[:], AF.Exp, scale=-0.5, bias=sbias[:]
                )
                invn2s[pair] = invn2

            def prep_norm(pair):
                # normalized+prescaled chunk into the resident fp8 tile
                c0 = pair * PAIR
                st = stages[pair]
                invn2 = invn2s.pop(pair)
                for k in range(KT):
                    nc.vector.tensor_mul(
                        n2f8[:, k, c0 : c0 + PAIR], st[:, k, :], invn2[:]
                    )
                # stage pair+4 reuses this pair's buffer: emit its load here
                # so the dispatch op enters the GpSimd FIFO only when it can
                # actually run (see note at the hoisted loads)
                if pair + 4 < NPAIR:
                    nc.gpsimd.dma_start(stages[pair + 4][:], f2s[pair + 4])

            def f1_norms():
                # ss1 = ||f1_i||^2 via DoubleRow ones-matmuls over squared
                # n1t8 (broadcast over partitions) + identity-mask extract.
                ss1t = pl.tile([128, 2048], FP32, tag="pslog", name="ss1bc")
                ss1bc = ss1t[:, 0:MC]
                for jk in range(KT // 2):
                    for h in range(2):
                        nc.tensor.matmul(
                            ss1bc[:, h * 512 : (h + 1) * 512],
                            ones8[:],
                            sq1[:, 2 * jk : 2 * jk + 2, h * 512 : (h + 1) * 512],
                            start=(jk == 0),
                            stop=(jk == KT // 2 - 1),
                            perf_mode=mybir.MatmulPerfMode.DoubleRow,
                        )
                for m in range(MT):
                    junk = ep.tile([128, 128], BF16, tag="junk", name="junk", bufs=2)
                    nc.vector.scalar_tensor_tensor(
                        junk[:],
                        ss1bc[:, m * 128 : (m + 1) * 128],
                        1.0,
                        eyeb[:],
                        op0=ALU.mult,
                        op1=ALU.mult,
                        accum_out=ss1x[:, m : m + 1],
                    )
                t1 = ep.tile([128, MT], FP32, tag="t1", name="t1", bufs=1)
                nc.scalar.activation(t1[:], ss1x[:], AF.Ln)
                # invn1T = 1 / (||f1_i|| * TEMP * SC)
                nc.scalar.activation(
                    invn1T[:], t1[:], AF.Exp, scale=-0.5, bias=lbias[:]
                )

            def main_pair(pair, weave=None, diag=False):
                # weave: {m_index: callback} emitted after that m's EXP so
                # every engine's FIFO queue stays in executable order
                c0 = pair * PAIR
                for m in range(MT):
                    ps2 = pl.tile([128, 2048], FP32, tag="pslog", name="pslog")
                    pslog = ps2[:, 0:PAIR]
                    for jk in range(KT // 2):
                        for h in range(2):
                            nc.tensor.matmul(
                                pslog[:, h * 512 : (h + 1) * 512],
                                n1t8[:, 2 * jk : 2 * jk + 2, m * 128 : (m + 1) * 128],
                                n2f8[
                                    :,
                                    2 * jk : 2 * jk + 2,
                                    c0 + h * 512 : c0 + (h + 1) * 512,
                                ],
                                start=(jk == 0),
                                stop=(jk == KT // 2 - 1),
                                perf_mode=mybir.MatmulPerfMode.DoubleRow,
                            )
                    # in-place EXP into the PSUM tile (exponentials are
                    # discarded; only accum_out matters) - saves the eb
                    # SBUF write traffic
                    col = m * QCOL + pair
                    nc.scalar.activation(
                        pslog[:],
                        pslog[:],
                        AF.Exp,
                        scale=invn1T[:, m : m + 1],
                        accum_out=spart[:, col : col + 1],
                    )
                    if diag:
                        # diagonal block of this core's rows lives in local
                        # pair 0 (host-side rotation): mask-extract the
                        # diagonal EXPONENTIAL (post-in-place-EXP; finalize
                        # takes Ln).  Extracting AFTER the EXP keeps the EXP
                        # free of any DVE dependency: Tile lowers cross-
                        # engine deps to per-engine counting semaphores, so
                        # an EXP->diag WAR made every EXP wait for ALL
                        # earlier DVE work incl. later pairs' prep
                        # (measured: first EXP at 58us instead of ~30us)
                        junk = ep.tile(
                            [128, 128], BF16, tag="junk", name="junk", bufs=2
                        )
                        nc.vector.scalar_tensor_tensor(
                            junk[:],
                            pslog[:, m * 128 : (m + 1) * 128],
                            1.0,
                            eyeb[:],
                            op0=ALU.mult,
                            op1=ALU.mult,
                            accum_out=draw[:, m : m + 1],
                        )
                    if weave and m in weave:
                        weave[m]()

            def main_group(pa, pb, gi, weave=None):
                # wide m-loop over two pairs: one [128,2048] ring tile and
                # ONE EXP per m covering both (halves the ACT per-op cost)
                for m in range(MT):
                    pslog = pl.tile([128, 2048], FP32, tag="pslog", name="pslog")
                    for half, pp_ in enumerate((pa, pb)):
                        hb = half * PAIR
                        c0 = pp_ * PAIR
                        for jk in range(KT // 2):
                            for h in range(2):
                                nc.tensor.matmul(
                                    pslog[:, hb + h * 512 : hb + (h + 1) * 512],
                                    n1t8[
                                        :,
                                        2 * jk : 2 * jk + 2,
                                        m * 128 : (m + 1) * 128,
                                    ],
                                    n2f8[
                                        :,
                                        2 * jk : 2 * jk + 2,
                                        c0 + h * 512 : c0 + (h + 1) * 512,
                                    ],
                                    start=(jk == 0),
                                    stop=(jk == KT // 2 - 1),
                                    perf_mode=mybir.MatmulPerfMode.DoubleRow,
                                )
                    col = m * QCOL + NLEG + gi
                    nc.scalar.activation(
                        pslog[:],
                        pslog[:],
                        AF.Exp,
                        scale=invn1T[:, m : m + 1],
                        accum_out=spart[:, col : col + 1],
                    )
                    if weave and m in weave:
                        weave[m]()

            # ---- startup: the f1-norm chain first in EVERY engine queue
            # (it gates all EXPs), then pair-0 prep.  high_priority pins it
            # ahead in the Tile scheduler's heaps: plain emission order was
            # NOT honored (t1-Ln measured at 60us, delaying every EXP).
            with tc.high_priority():
                f1_sq()
                f1_norms()
                prep_sq(0, pool_chunks=2)
                prep_ss(0)
                prep_norm(0)
            # pair 0 weaves pair-1 prep and pair-2 squares; each pair's
            # ss/norm is woven one main LATER than its squares: the ss
            # ring tile's WAR then anchors its Ln's placement BEHIND the
            # preceding EXPs in the Scalar stream.  (The scheduler orders
            # streams by simulated dispatch time and its DMA model is
            # optimistic for queued stages, so earlier-woven Lns were
            # placed ahead of main0's EXPs and head-of-line blocked them:
            # first EXP measured 42.5us with deps ready at ~25us.)
            main_pair(
                0,
                diag=True,
                weave={
                    0: lambda: prep_sq(1, pool_chunks=1),
                    2: lambda: prep_ss(1),
                    3: lambda: prep_norm(1),
                    5: lambda: prep_sq(2),
                },
            )
            # draw holds exp(l_ii * s_i): dvals = ln(draw) = scaled diag
            nc.scalar.activation(dvals[:], draw[:], AF.Ln)
            main_pair(
                1,
                weave={
                    0: lambda: prep_ss(2),
                    1: lambda: prep_norm(2),
                    3: lambda: prep_sq(3),
                },
            )
            main_pair(
                2,
                weave={
                    0: lambda: prep_ss(3),
                    1: lambda: prep_norm(3),
                    2: lambda: prep_sq(4),
                    4: lambda: prep_ss(4),
                    6: lambda: prep_norm(4),
                },
            )
            main_pair(
                3,
                weave={
                    0: lambda: prep_sq(5),
                    3: lambda: prep_ss(5),
                    5: lambda: prep_norm(5),
                },
            )
            # pairs 4-7 as two wide groups.  Group (4,5) weaves preps for
            # 6 and 7; its m-slots are 3.4us apart (12 matmuls each) so the
            # ss ones-matmuls sit in the PE FIFO only after their squares
            # are provably done.
            main_group(
                4, 5, 0,
                weave={
                    0: lambda: prep_sq(6),
                    1: lambda: prep_sq(7),
                    3: lambda: prep_ss(6),
                    5: lambda: prep_norm(6),
                    6: lambda: prep_ss(7),
                    7: lambda: prep_norm(7),
                },
            )
            main_group(6, 7, 1)

            # ---- finalize ----
            S = ep.tile([128, MT], FP32, tag="S", name="S", bufs=1)
            nc.vector.reduce_sum(
                S[:], spart[:].rearrange("p (m q) -> p m q", q=QCOL), axis=AX.X
            )
            lse = ep.tile([128, MT], FP32, tag="lse", name="lse", bufs=1)
            nc.scalar.activation(lse[:], S[:], AF.Ln)
            res = ep.tile([128, MT], FP32, tag="res", name="res", bufs=1)
            nc.vector.tensor_sub(res[:], lse[:], dvals[:])
            nc.sync.dma_start(out[:, :], res[:])

    _split_excess_waits(nc)
    return nc


def make_in_maps(f1: np.ndarray, f2: np.ndarray) -> list[dict[str, np.ndarray]]:
    f1 = np.ascontiguousarray(np.asarray(f1, dtype=np.float32))
    f2 = np.ascontiguousarray(np.asarray(f2, dtype=np.float32))
    assert f1.shape == (N, D) and f2.shape == (N, D)
    f2t = f2.T  # [D, N]
    # pack pair-major, partition-major: f2s[q, p, k, n] = f2t[k*128+p, q*1024+n]
    # pre-cast to bf16 on the host: the device consumes f1/f2 at
    # bf16/fp8 precision anyway (the SWDGE load already cast fp32->bf16),
    # and halving the HBM bytes halves the stage-DMA cadence that gates
    # the startup pipe-fill (stage p landed at ~17.6+8.8p us)
    import ml_dtypes

    bf16 = ml_dtypes.bfloat16
    f2s = np.ascontiguousarray(
        f2t.reshape(KT, 128, NPAIR, PAIR).transpose(2, 1, 0, 3).astype(bf16)
    )
    eye = np.ascontiguousarray(np.eye(128, dtype=np.float32))
    in_maps = []
    for c in range(NCORES):
        f1c = np.ascontiguousarray(f1[c * MC : (c + 1) * MC])
        # rotate pairs so the diagonal block is local pair 0 on every core
        f2sc = np.ascontiguousarray(np.roll(f2s, -c, axis=0))
        in_maps.append(
            {
                "f1t": np.ascontiguousarray(f1c.T.astype(bf16)),
                "f2s": f2sc,
                "eye": eye,
            }
        )
    return in_maps


def combine_outputs(outs: list[np.ndarray]) -> np.float32:
    total = 0.0
    for o in outs:
        total += float(np.sum(np.asarray(o, dtype=np.float64)))
    return np.float32(total / float(N))


def run(f1: np.ndarray, f2: np.ndarray, trace: bool = False):
    from concourse.bass_utils import run_bass_kernel_spmd

    nc = build_program()
    in_maps = make_in_maps(f1, f2)
    r = run_bass_kernel_spmd(nc, in_maps, core_ids=list(range(NCORES)), trace=trace)
    outs = [m["out"] for m in r.results]
    return combine_outputs(outs), r


def kernel(f1: np.ndarray, f2: np.ndarray) -> np.ndarray:
    loss, _ = run(f1, f2, trace=False)
    return loss


if __name__ == "__main__":
    f1 = np.random.randn(N, D).astype(np.float32)
    f2 = np.random.randn(N, D).astype(np.float32)
    print(kernel(f1, f2))

